# revision 19
# baseline (speedup 1.0000x reference)
"""Augmented-memory scaled-dot-product attention on 8 Trainium2 NeuronCores.

Sharding: data-parallel over batch (B=8 -> 1 batch element per core), weights
replicated.  Inside each core everything is computed in a transposed layout:

  qT[hd, q]   = Wq^T @ x_q^T          (projections, contraction over d_model)
  kT[hd, k]   = Wk^T @ x_k^T  (+ memory slots appended as columns)
  v[k, hd]    = x_v^T as lhsT @ Wv    (natural layout, via DRAM, memory slots
                                       kept on-chip)
  sT[k, q]    = kT_h-slice^T @ qT_h   (scores, transposed: k on partitions)
  eT[k, q]    = exp(scale * sT) * notmask          (masked unnormalized)
  sums[q]     = ones^T @ eT           (softmax denominator via PE ones-matmul)
  oT[dv, q]   = v_h-slice^T @ eT, normalized by broadcast(1/sums)
  out[q, d]   = oT-slice^T @ Wo (+ bo)

The attention-probability output is returned transposed+unnormalized
(eT and sums); the host divides during the transpose it has to do anyway.
All matmul operands use float32r (full PE rate; same bits as float32).
"""

import math

import numpy as np

B, NQ, NK, D_MODEL, H, D_K, D_V, M = 8, 1024, 1024, 1024, 8, 128, 128, 16
NKM = NK + M          # 1040
P = 128
DO = D_MODEL // P     # 8 d-tiles
KO = NK // P          # 8 full k-tiles
CH = 2                # nq chunks of 512
CW = 512
SCALE = 1.0 / math.sqrt(D_K)

_cached = {}


def _split_excess_waits(nc):
    """This walrus build rejects >1 sync-wait per instruction. Move excess
    SyncWaits onto same-engine NOPs inserted directly before the owner."""
    import concourse.mybir as mybir

    for f in nc.m.functions:
        for bb in f.blocks:
            out, changed = [], False
            for inst in bb.instructions:
                si = inst.sync_info
                if si is not None and si.on_wait and len(si.on_wait) > 1:
                    waits = list(si.on_wait)
                    for w in waits[:-1]:
                        out.append(
                            mybir.InstNoOp(
                                name=nc.get_next_instruction_name(),
                                sync_info=mybir.SyncInfo(on_wait=[w], on_update=[]),
                                bass_nofuse=True,
                                engine=inst.engine,
                            )
                        )
                    si.on_wait = waits[-1:]
                    changed = True
                out.append(inst)
            if changed:
                bb.instructions = out
    return nc


def _build_program(with_bias):
    import concourse.bass as bass
    import concourse.mybir as mybir
    import concourse.tile as tile

    f32 = mybir.dt.float32
    f32r = mybir.dt.float32r
    u8 = mybir.dt.uint8
    EXP = mybir.ActivationFunctionType.Exp
    COPY = mybir.ActivationFunctionType.Copy
    MUL = mybir.AluOpType.mult

    nc = bass.Bass()
    dp = nc.declare_dram_parameter
    xqT = dp("xqT", [D_MODEL, NQ], f32r, isOutput=False)
    xkT = dp("xkT", [D_MODEL, NK], f32r, isOutput=False)
    xvT = dp("xvT", [D_MODEL, NK], f32r, isOutput=False)
    wq = dp("wq", [D_MODEL, H * D_K], f32r, isOutput=False)
    wk = dp("wk", [D_MODEL, H * D_K], f32r, isOutput=False)
    wv = dp("wv", [D_MODEL, H * D_V], f32r, isOutput=False)
    wo = dp("wo", [H * D_V, D_MODEL], f32r, isOutput=False)
    mkT = dp("mkT", [H, D_K, M], f32r, isOutput=False)   # sqrt(dk)*m_k, [h][dk][slot]
    mv = dp("mv", [M, H * D_V], f32r, isOutput=False)    # sqrt(M)*m_v
    nmT = dp("nmT", [H, NK, NQ], u8, isOutput=False)     # ~mask, [h][k][q]
    if with_bias:
        bqp = dp("bq", [P, DO], f32, isOutput=False)     # partition-major
        bkp = dp("bk", [P, DO], f32, isOutput=False)
        bvp = dp("bv", [1, H * D_V], f32r, isOutput=False)
        bop = dp("bo", [1, D_MODEL], f32r, isOutput=False)
    out_d = dp("out", [NQ, D_MODEL], f32r, isOutput=True)
    attT_d = dp("attT", [H, NKM, NQ], f32r, isOutput=True)
    sums_d = dp("sums", [H, NQ], f32r, isOutput=True)

    with tile.TileContext(nc) as tc:
        with tc.tile_pool(name="persist", bufs=1) as pers:
            qT = pers.tile([P, H, NQ], f32r, name="qT")
            kT = pers.tile([P, H, NKM], f32r, name="kT")
            vv = pers.tile([P, KO, H * D_V], f32r, name="vv")
            vmem = pers.tile([M, H * D_V], f32r, name="vmem")
            onesK = pers.tile([P, 1], f32r, name="onesK")
            onesM = pers.tile([M, 1], f32r, name="onesM")
            ones1 = pers.tile([1, P], f32r, name="ones1")
            ones_f = pers.tile([P, 1], f32, name="ones_f")
            nc.vector.memset(ones_f[:], 1.0)
            nc.vector.tensor_copy(onesK[:], ones_f[:])
            nc.vector.tensor_copy(onesM[:], ones_f[:M, :])
            one_row_f = pers.tile([1, P], f32, name="one_row_f")
            nc.vector.memset(one_row_f[:], 1.0)
            nc.vector.tensor_copy(ones1[:], one_row_f[:])
            if with_bias:
                bq_t = pers.tile([P, DO], f32, name="bq_t")
                bk_t = pers.tile([P, DO], f32, name="bk_t")
                bv_t = pers.tile([1, H * D_V], f32r, name="bv_t")
                bo_t = pers.tile([1, D_MODEL], f32r, name="bo_t")
                nc.sync.dma_start(bq_t[:], bqp[:])
                nc.sync.dma_start(bk_t[:], bkp[:])
                nc.sync.dma_start(bv_t[:], bvp[:])
                nc.sync.dma_start(bo_t[:], bop[:])

            # ------------- phase 1: projections (c-outer, x halves) -------------
            with tc.tile_pool(name="xpool", bufs=2) as xpool, \
                 tc.tile_pool(name="wpool", bufs=2) as wpool, \
                 tc.tile_pool(name="pjps", bufs=3, space="PSUM") as pjps:
                # for q/k the weight is stationary (full) and x streams in
                # column halves; for v the roles swap: xvT is stationary (full)
                # and Wv streams in hd-halves. Same indexing either way.
                for pi, (big, small) in enumerate(
                        [(wq, xqT), (wk, xkT), (xvT, wv)]):
                    wt = wpool.tile([P, DO, H * D_K], f32r, tag="w")
                    nc.sync.dma_start(wt[:], big.rearrange("(o p) n -> p o n", p=P))
                    for c in range(CH):
                        cs = slice(c * CW, (c + 1) * CW)
                        xt = xpool.tile([P, DO, CW], f32r, tag="x")
                        nc.sync.dma_start(
                            xt[:], small[:, cs].rearrange("(o p) n -> p o n", p=P))
                        for m in range(DO):
                            ps = pjps.tile([P, CW], f32, tag="pj")
                            for d in range(DO):
                                lhsT = wt[:, d, m * P:(m + 1) * P]
                                rhs = xt[:, d, :]
                                nc.tensor.matmul(
                                    ps[:], lhsT, rhs,
                                    start=(d == 0),
                                    stop=(d == DO - 1 and not (with_bias and pi == 2)))
                            if pi == 0:
                                bias = bq_t[:, m:m + 1] if with_bias else 0.0
                                nc.scalar.activation(
                                    qT[:, m, cs], ps[:], COPY, bias=bias)
                            elif pi == 1:
                                bias = bk_t[:, m:m + 1] if with_bias else 0.0
                                nc.scalar.activation(
                                    kT[:, m, cs], ps[:], COPY, bias=bias)
                            else:
                                if with_bias:
                                    nc.tensor.matmul(
                                        ps[:], ones1[:], bv_t[:, cs],
                                        start=False, stop=True)
                                nc.vector.tensor_copy(vv[:, m, cs], ps[:])
                # memory slots
                for h in range(H):
                    nc.sync.dma_start(kT[:, h, NK:NKM], mkT[h])
                nc.sync.dma_start(vmem[:], mv[:])

            # ---------------- phase 2: per-head attention ----------------
            with tc.tile_pool(name="otp", bufs=1) as otp:
                oT = otp.tile([P, H, NQ], f32r, name="oT")
                with tc.tile_pool(name="expp", bufs=10) as expp, \
                     tc.tile_pool(name="emp", bufs=2) as emp, \
                     tc.tile_pool(name="nmp", bufs=3) as nmp, \
                     tc.tile_pool(name="rcp", bufs=2) as rcp, \
                     tc.tile_pool(name="smp", bufs=2) as smp, \
                     tc.tile_pool(name="lnp", bufs=2) as lnp, \
                     tc.tile_pool(name="sps", bufs=4, space="PSUM") as sps, \
                     tc.tile_pool(name="bcps", bufs=1, space="PSUM") as bcps, \
                     tc.tile_pool(name="smps", bufs=1, space="PSUM") as smps, \
                     tc.tile_pool(name="avps", bufs=2, space="PSUM") as avps:
                    pending = None
                    LAG = 3

                    def finish(hp, sums_hp):
                        for c in range(CH):
                            cs = slice(c * CW, (c + 1) * CW)
                            bps = bcps.tile([P, CW], f32, tag="bc")
                            nc.tensor.matmul(
                                bps[:], ones1[:], sums_hp[:, cs],
                                start=True, stop=True)
                            rc = rcp.tile([P, CW], f32, tag="rc")
                            nc.vector.reciprocal(rc[:], bps[:])
                            nc.vector.tensor_tensor(
                                oT[:, hp, cs], oT[:, hp, cs], rc[:], MUL)

                    for h in range(H):
                        exp_tiles = []
                        nsteps = KO + 1    # 8 k-tiles + memory-slot tile
                        sums_c = [None, None]
                        av_c = [None, None]
                        sums_sb = smp.tile([1, NQ], f32r, tag="sumsb")

                        def emit_score(t):
                            if t < KO:
                                k = t
                                nmt = nmp.tile([P, NQ], u8, tag="nm",
                                               name=f"nm_{h}_{k}")
                                nc.sync.dma_start(
                                    nmt[:], nmT[h, k * P:(k + 1) * P, :])
                                et = expp.tile([P, NQ], f32r, tag="exp",
                                               name=f"exp_{h}_{k}")
                                for c in range(CH):
                                    cs = slice(c * CW, (c + 1) * CW)
                                    ps = sps.tile([P, CW], f32, tag="s",
                                                  name=f"s_{h}_{k}_{c}")
                                    nc.tensor.matmul(
                                        ps[:], kT[:, h, k * P:(k + 1) * P],
                                        qT[:, h, cs], start=True, stop=True)
                                    nc.scalar.activation(
                                        et[:, cs], ps[:], EXP, scale=SCALE)
                                nc.vector.tensor_tensor(et[:], et[:], nmt[:], MUL)
                                nc.gpsimd.dma_start(
                                    attT_d[h, k * P:(k + 1) * P, :], et[:])
                                exp_tiles.append(et)
                            else:
                                em = emp.tile([M, NQ], f32r, tag="em",
                                              name=f"em_{h}")
                                for c in range(CH):
                                    cs = slice(c * CW, (c + 1) * CW)
                                    psm = sps.tile([M, CW], f32, tag="s",
                                                   name=f"sm_{h}_{c}")
                                    nc.tensor.matmul(
                                        psm[:], kT[:, h, NK:NKM], qT[:, h, cs],
                                        start=True, stop=True)
                                    nc.scalar.activation(
                                        em[:, cs], psm[:], EXP, scale=SCALE)
                                nc.gpsimd.dma_start(attT_d[h, NK:NKM, :], em[:])
                                exp_tiles.append(em)

                        def emit_sumav(c, t):
                            cs = slice(c * CW, (c + 1) * CW)
                            if t == 0:
                                sums_c[c] = smps.tile([1, CW], f32, tag="sum",
                                                      name=f"sum_{h}_{c}")
                                av_c[c] = avps.tile([P, CW], f32, tag="av",
                                                    name=f"av_{h}_{c}")
                            lhs_s = onesK if t < KO else onesM
                            lhs_v = (vv[:, t, h * P:(h + 1) * P] if t < KO
                                     else vmem[:, h * P:(h + 1) * P])
                            nc.tensor.matmul(
                                sums_c[c][:], lhs_s[:], exp_tiles[t][:, cs],
                                start=(t == 0), stop=(t == nsteps - 1))
                            nc.tensor.matmul(
                                av_c[c][:], lhs_v, exp_tiles[t][:, cs],
                                start=(t == 0), stop=(t == nsteps - 1))
                            if t == nsteps - 1:
                                nc.vector.tensor_copy(sums_sb[:, cs], sums_c[c][:])
                                nc.scalar.activation(oT[:, h, cs], av_c[c][:], COPY)

                        # lag-interleaved: PE consumes masked exp at the rate
                        # ACT/DVE produce it, no lumpy waits
                        for t in range(nsteps):
                            emit_score(t)
                            if t == nsteps - 1 and pending is not None:
                                finish(*pending)
                            if t >= LAG:
                                emit_sumav(0, t - LAG)
                        for t in range(nsteps - LAG, nsteps):
                            emit_sumav(0, t)
                        for t in range(nsteps):
                            emit_sumav(1, t)
                        nc.gpsimd.dma_start(sums_d[h:h + 1, :], sums_sb[:])
                        pending = (h, sums_sb)

                    if pending is not None:
                        finish(*pending)

                # ------------ phase 3: output projection ------------
                with tc.tile_pool(name="wop", bufs=1) as wop, \
                     tc.tile_pool(name="fop", bufs=3) as fop, \
                     tc.tile_pool(name="fps", bufs=2, space="PSUM") as fps:
                    wot = wop.tile([P, H, D_MODEL], f32r, name="wot")
                    nc.sync.dma_start(wot[:], wo.rearrange("(o p) n -> p o n", p=P))
                    for m in range(DO):
                        for c in range(CH):
                            cs = slice(c * CW, (c + 1) * CW)
                            ps = fps.tile([P, CW], f32, tag="f")
                            for hh in range(H):
                                nc.tensor.matmul(
                                    ps[:], oT[:, hh, m * P:(m + 1) * P],
                                    wot[:, hh, cs], start=(hh == 0),
                                    stop=(hh == H - 1 and not with_bias))
                            if with_bias:
                                nc.tensor.matmul(
                                    ps[:], ones1[:], bo_t[:, cs],
                                    start=False, stop=True)
                            ot = fop.tile([P, CW], f32r, tag="fo")
                            nc.scalar.activation(ot[:], ps[:], COPY)
                            nc.gpsimd.dma_start(
                                out_d[m * P:(m + 1) * P, cs], ot[:])
    return _split_excess_waits(nc)


def kernel(queries, keys, values, attention_mask, Wq, bq, Wk, bk, Wv, bv,
           Wo, bo, m_k, m_v):
    from concourse.bass_utils import run_bass_kernel_spmd

    queries = np.asarray(queries, dtype=np.float32)
    keys = np.asarray(keys, dtype=np.float32)
    values = np.asarray(values, dtype=np.float32)
    mask = np.asarray(attention_mask)
    Wq, Wk, Wv, Wo = (np.asarray(w, dtype=np.float32) for w in (Wq, Wk, Wv, Wo))
    bq, bk, bv, bo = (np.asarray(b, dtype=np.float32) for b in (bq, bk, bv, bo))
    m_k = np.asarray(m_k, dtype=np.float32)
    m_v = np.asarray(m_v, dtype=np.float32)

    with_bias = any(np.any(b) for b in (bq, bk, bv, bo))
    key = ("prog", with_bias)
    if key not in _cached:
        _cached[key] = _build_program(with_bias)
    nc = _cached[key]

    mkT_all = np.ascontiguousarray(
        (math.sqrt(D_K) * m_k[0].T).reshape(H, D_K, M))
    mv_all = np.ascontiguousarray(math.sqrt(M) * m_v[0])
    notmask = np.logical_not(mask)

    in_maps = []
    for b in range(B):
        im = {
            "xqT": np.ascontiguousarray(queries[b].T),
            "xkT": np.ascontiguousarray(keys[b].T),
            "xvT": np.ascontiguousarray(values[b].T),
            "wq": Wq, "wk": Wk, "wv": Wv, "wo": Wo,
            "mkT": mkT_all, "mv": mv_all,
            "nmT": np.ascontiguousarray(
                notmask[b].transpose(0, 2, 1)).view(np.uint8),
        }
        if with_bias:
            im["bq"] = np.ascontiguousarray(bq.reshape(DO, P).T)
            im["bk"] = np.ascontiguousarray(bk.reshape(DO, P).T)
            im["bv"] = bv.reshape(1, H * D_V)
            im["bo"] = bo.reshape(1, D_MODEL)
        in_maps.append(im)

    res = run_bass_kernel_spmd(nc, in_maps, list(range(B)))

    out = np.empty((B, NQ, D_MODEL), dtype=np.float32)
    att = np.empty((B, H, NQ, NKM), dtype=np.float32)
    for b in range(B):
        r = res.results[b]
        out[b] = r["out"]
        np.divide(r["attT"].transpose(0, 2, 1), r["sums"][:, :, None],
                  out=att[b])
    return out, att.reshape(-1, NQ, NK)


# revision 21
# speedup vs baseline: 1.0208x; 1.0208x over previous
"""Augmented-memory scaled-dot-product attention on 8 Trainium2 NeuronCores.

Sharding: data-parallel over batch (B=8 -> 1 batch element per core), weights
replicated.  Inside each core everything is computed in a transposed layout:

  qT[hd, q]   = Wq^T @ x_q^T          (projections, contraction over d_model)
  kT[hd, k]   = Wk^T @ x_k^T  (+ memory slots appended as columns)
  v[k, hd]    = x_v^T as lhsT @ Wv    (natural layout, via DRAM, memory slots
                                       kept on-chip)
  sT[k, q]    = kT_h-slice^T @ qT_h   (scores, transposed: k on partitions)
  eT[k, q]    = exp(scale * sT) * notmask          (masked unnormalized)
  sums[q]     = ones^T @ eT           (softmax denominator via PE ones-matmul)
  oT[dv, q]   = v_h-slice^T @ eT, normalized by broadcast(1/sums)
  out[q, d]   = oT-slice^T @ Wo (+ bo)

The attention-probability output is returned transposed+unnormalized
(eT and sums); the host divides during the transpose it has to do anyway.
All matmul operands use float32r (full PE rate; same bits as float32).
"""

import math

import numpy as np

B, NQ, NK, D_MODEL, H, D_K, D_V, M = 8, 1024, 1024, 1024, 8, 128, 128, 16
NKM = NK + M          # 1040
P = 128
DO = D_MODEL // P     # 8 d-tiles
KO = NK // P          # 8 full k-tiles
CH = 2                # nq chunks of 512
CW = 512
SCALE = 1.0 / math.sqrt(D_K)

_cached = {}


def _split_excess_waits(nc):
    """This walrus build rejects >1 sync-wait per instruction. Move excess
    SyncWaits onto same-engine NOPs inserted directly before the owner."""
    import concourse.mybir as mybir

    for f in nc.m.functions:
        for bb in f.blocks:
            out, changed = [], False
            for inst in bb.instructions:
                si = inst.sync_info
                if si is not None and si.on_wait and len(si.on_wait) > 1:
                    waits = list(si.on_wait)
                    for w in waits[:-1]:
                        out.append(
                            mybir.InstNoOp(
                                name=nc.get_next_instruction_name(),
                                sync_info=mybir.SyncInfo(on_wait=[w], on_update=[]),
                                bass_nofuse=True,
                                engine=inst.engine,
                            )
                        )
                    si.on_wait = waits[-1:]
                    changed = True
                out.append(inst)
            if changed:
                bb.instructions = out
    return nc


def _build_program(with_bias):
    import concourse.bass as bass
    import concourse.mybir as mybir
    import concourse.tile as tile

    f32 = mybir.dt.float32
    f32r = mybir.dt.float32r
    u8 = mybir.dt.uint8
    EXP = mybir.ActivationFunctionType.Exp
    COPY = mybir.ActivationFunctionType.Copy
    MUL = mybir.AluOpType.mult

    nc = bass.Bass()
    dp = nc.declare_dram_parameter
    xqT = dp("xqT", [D_MODEL, NQ], f32r, isOutput=False)
    xkT = dp("xkT", [D_MODEL, NK], f32r, isOutput=False)
    xvT = dp("xvT", [D_MODEL, NK], f32r, isOutput=False)
    wq = dp("wq", [D_MODEL, H * D_K], f32r, isOutput=False)
    wk = dp("wk", [D_MODEL, H * D_K], f32r, isOutput=False)
    wv = dp("wv", [D_MODEL, H * D_V], f32r, isOutput=False)
    wo = dp("wo", [H * D_V, D_MODEL], f32r, isOutput=False)
    mkT = dp("mkT", [H, D_K, M], f32r, isOutput=False)   # sqrt(dk)*m_k, [h][dk][slot]
    mv = dp("mv", [M, H * D_V], f32r, isOutput=False)    # sqrt(M)*m_v
    nmT = dp("nmT", [H, NK, NQ], u8, isOutput=False)     # ~mask, [h][k][q]
    if with_bias:
        bqp = dp("bq", [P, DO], f32, isOutput=False)     # partition-major
        bkp = dp("bk", [P, DO], f32, isOutput=False)
        bvp = dp("bv", [1, H * D_V], f32r, isOutput=False)
        bop = dp("bo", [1, D_MODEL], f32r, isOutput=False)
    out_d = dp("out", [NQ, D_MODEL], f32r, isOutput=True)
    attT_d = dp("attT", [H, NKM, NQ], f32r, isOutput=True)
    sums_d = dp("sums", [H, NQ], f32r, isOutput=True)

    with tile.TileContext(nc) as tc:
        with tc.tile_pool(name="persist", bufs=1) as pers:
            qT = pers.tile([P, H, NQ], f32r, name="qT")
            kT = pers.tile([P, H, NKM], f32r, name="kT")
            vv = pers.tile([P, KO, H * D_V], f32r, name="vv")
            vmem = pers.tile([M, H * D_V], f32r, name="vmem")
            onesK = pers.tile([P, 1], f32r, name="onesK")
            onesM = pers.tile([M, 1], f32r, name="onesM")
            ones1 = pers.tile([1, P], f32r, name="ones1")
            ones_f = pers.tile([P, 1], f32, name="ones_f")
            nc.vector.memset(ones_f[:], 1.0)
            nc.vector.tensor_copy(onesK[:], ones_f[:])
            nc.vector.tensor_copy(onesM[:], ones_f[:M, :])
            one_row_f = pers.tile([1, P], f32, name="one_row_f")
            nc.vector.memset(one_row_f[:], 1.0)
            nc.vector.tensor_copy(ones1[:], one_row_f[:])
            if with_bias:
                bq_t = pers.tile([P, DO], f32, name="bq_t")
                bk_t = pers.tile([P, DO], f32, name="bk_t")
                bv_t = pers.tile([1, H * D_V], f32r, name="bv_t")
                bo_t = pers.tile([1, D_MODEL], f32r, name="bo_t")
                nc.sync.dma_start(bq_t[:], bqp[:])
                nc.sync.dma_start(bk_t[:], bkp[:])
                nc.sync.dma_start(bv_t[:], bvp[:])
                nc.sync.dma_start(bo_t[:], bop[:])

            # ------------- phase 1: projections (c-outer, x halves) -------------
            with tc.tile_pool(name="xpool", bufs=2) as xpool, \
                 tc.tile_pool(name="wpool", bufs=2) as wpool, \
                 tc.tile_pool(name="pjps", bufs=3, space="PSUM") as pjps:
                # for q/k the weight is stationary (full) and x streams in
                # column halves; for v the roles swap: xvT is stationary (full)
                # and Wv streams in hd-halves. Same indexing either way.
                for pi, (big, small) in enumerate(
                        [(wq, xqT), (wk, xkT), (xvT, wv)]):
                    wt = wpool.tile([P, DO, H * D_K], f32r, tag="w")
                    nc.sync.dma_start(wt[:], big.rearrange("(o p) n -> p o n", p=P))
                    for c in range(CH):
                        cs = slice(c * CW, (c + 1) * CW)
                        xt = xpool.tile([P, DO, CW], f32r, tag="x")
                        nc.sync.dma_start(
                            xt[:], small[:, cs].rearrange("(o p) n -> p o n", p=P))
                        for m in range(DO):
                            ps = pjps.tile([P, CW], f32, tag="pj")
                            for d in range(DO):
                                lhsT = wt[:, d, m * P:(m + 1) * P]
                                rhs = xt[:, d, :]
                                nc.tensor.matmul(
                                    ps[:], lhsT, rhs,
                                    start=(d == 0),
                                    stop=(d == DO - 1 and not (with_bias and pi == 2)))
                            if pi == 0:
                                bias = bq_t[:, m:m + 1] if with_bias else 0.0
                                nc.scalar.activation(
                                    qT[:, m, cs], ps[:], COPY, bias=bias)
                            elif pi == 1:
                                bias = bk_t[:, m:m + 1] if with_bias else 0.0
                                nc.scalar.activation(
                                    kT[:, m, cs], ps[:], COPY, bias=bias)
                            else:
                                if with_bias:
                                    nc.tensor.matmul(
                                        ps[:], ones1[:], bv_t[:, cs],
                                        start=False, stop=True)
                                nc.vector.tensor_copy(vv[:, m, cs], ps[:])
                # memory slots
                for h in range(H):
                    nc.sync.dma_start(kT[:, h, NK:NKM], mkT[h])
                nc.sync.dma_start(vmem[:], mv[:])

            # ---------------- phase 2: per-head attention ----------------
            with tc.tile_pool(name="otp", bufs=1) as otp:
                oT = otp.tile([P, H, NQ], f32r, name="oT")
                with tc.tile_pool(name="expp", bufs=10) as expp, \
                     tc.tile_pool(name="emp", bufs=2) as emp, \
                     tc.tile_pool(name="nmp", bufs=3) as nmp, \
                     tc.tile_pool(name="rcp", bufs=1) as rcp, \
                     tc.tile_pool(name="smp", bufs=2) as smp, \
                     tc.tile_pool(name="sps", bufs=4, space="PSUM") as sps, \
                     tc.tile_pool(name="bcps", bufs=1, space="PSUM") as bcps, \
                     tc.tile_pool(name="smps", bufs=1, space="PSUM") as smps, \
                     tc.tile_pool(name="avps", bufs=2, space="PSUM") as avps:
                    pending = None
                    LAG = 3

                    def finish(hp, sums_hp, ln_hp):
                        for c in range(CH):
                            cs = slice(c * CW, (c + 1) * CW)
                            bps = bcps.tile([P, CW], f32, tag="bc")
                            nc.tensor.matmul(
                                bps[:], ones1[:], ln_hp[:, cs],
                                start=True, stop=True)
                            rc0 = rcp.tile([P, CW], f32, tag="rc0")
                            nc.scalar.activation(rc0[:], bps[:], EXP, scale=-1.0)
                            # one Newton-Raphson step: rc = rc0*(2 - s*rc0).
                            # Computed negated ((s*rc0 - 2)*rc0); cancels with
                            # the negated oT eviction.
                            bps2 = bcps.tile([P, CW], f32, tag="bc")
                            nc.tensor.matmul(
                                bps2[:], ones1[:], sums_hp[:, cs],
                                start=True, stop=True)
                            u = rcp.tile([P, CW], f32, tag="u")
                            nc.vector.tensor_tensor(u[:], bps2[:], rc0[:], MUL)
                            rc = rcp.tile([P, CW], f32, tag="rc")
                            nc.vector.scalar_tensor_tensor(
                                out=rc[:], in0=u[:], scalar=2.0, in1=rc0[:],
                                op0=mybir.AluOpType.subtract, op1=MUL)
                            nc.vector.tensor_tensor(
                                oT[:, hp, cs], oT[:, hp, cs], rc[:], MUL)

                    for h in range(H):
                        exp_tiles = []
                        nsteps = KO + 1    # 8 k-tiles + memory-slot tile
                        sums_c = [None, None]
                        av_c = [None, None]
                        sums_sb = smp.tile([1, NQ], f32r, tag="sumsb")
                        lnt = smp.tile([1, NQ], f32r, tag="lnt")

                        def emit_score(t):
                            if t < KO:
                                k = t
                                nmt = nmp.tile([P, NQ], u8, tag="nm",
                                               name=f"nm_{h}_{k}")
                                nc.sync.dma_start(
                                    nmt[:], nmT[h, k * P:(k + 1) * P, :])
                                et = expp.tile([P, NQ], f32r, tag="exp",
                                               name=f"exp_{h}_{k}")
                                for c in range(CH):
                                    cs = slice(c * CW, (c + 1) * CW)
                                    ps = sps.tile([P, CW], f32, tag="s",
                                                  name=f"s_{h}_{k}_{c}")
                                    nc.tensor.matmul(
                                        ps[:], kT[:, h, k * P:(k + 1) * P],
                                        qT[:, h, cs], start=True, stop=True)
                                    nc.scalar.activation(
                                        et[:, cs], ps[:], EXP, scale=SCALE)
                                nc.vector.tensor_tensor(et[:], et[:], nmt[:], MUL)
                                nc.gpsimd.dma_start(
                                    attT_d[h, k * P:(k + 1) * P, :], et[:])
                                exp_tiles.append(et)
                            else:
                                em = emp.tile([M, NQ], f32r, tag="em",
                                              name=f"em_{h}")
                                for c in range(CH):
                                    cs = slice(c * CW, (c + 1) * CW)
                                    psm = sps.tile([M, CW], f32, tag="s",
                                                   name=f"sm_{h}_{c}")
                                    nc.tensor.matmul(
                                        psm[:], kT[:, h, NK:NKM], qT[:, h, cs],
                                        start=True, stop=True)
                                    nc.scalar.activation(
                                        em[:, cs], psm[:], EXP, scale=SCALE)
                                nc.gpsimd.dma_start(attT_d[h, NK:NKM, :], em[:])
                                exp_tiles.append(em)

                        def emit_sumav(c, t):
                            cs = slice(c * CW, (c + 1) * CW)
                            if t == 0:
                                sums_c[c] = smps.tile([1, CW], f32, tag="sum",
                                                      name=f"sum_{h}_{c}")
                                av_c[c] = avps.tile([P, CW], f32, tag="av",
                                                    name=f"av_{h}_{c}")
                            lhs_s = onesK if t < KO else onesM
                            lhs_v = (vv[:, t, h * P:(h + 1) * P] if t < KO
                                     else vmem[:, h * P:(h + 1) * P])
                            nc.tensor.matmul(
                                sums_c[c][:], lhs_s[:], exp_tiles[t][:, cs],
                                start=(t == 0), stop=(t == nsteps - 1))
                            nc.tensor.matmul(
                                av_c[c][:], lhs_v, exp_tiles[t][:, cs],
                                start=(t == 0), stop=(t == nsteps - 1))
                            if t == nsteps - 1:
                                nc.vector.tensor_copy(sums_sb[:, cs], sums_c[c][:])
                                nc.scalar.activation(
                                    lnt[:, cs], sums_sb[:, cs],
                                    mybir.ActivationFunctionType.Ln)
                                nc.scalar.activation(
                                    oT[:, h, cs], av_c[c][:], COPY, scale=-1.0)

                        # lag-interleaved: PE consumes masked exp at the rate
                        # ACT/DVE produce it, no lumpy waits
                        for t in range(nsteps):
                            emit_score(t)
                            if t == nsteps - 1 and pending is not None:
                                finish(*pending)
                            if t >= LAG:
                                emit_sumav(0, t - LAG)
                        for t in range(nsteps - LAG, nsteps):
                            emit_sumav(0, t)
                        for t in range(nsteps):
                            emit_sumav(1, t)
                        nc.gpsimd.dma_start(sums_d[h:h + 1, :], sums_sb[:])
                        pending = (h, sums_sb, lnt)

                    if pending is not None:
                        finish(*pending)

                # ------------ phase 3: output projection ------------
                with tc.tile_pool(name="wop", bufs=2) as wop, \
                     tc.tile_pool(name="fop", bufs=3) as fop, \
                     tc.tile_pool(name="fps", bufs=2, space="PSUM") as fps:
                    for c in range(CH):
                        cs = slice(c * CW, (c + 1) * CW)
                        wot = wop.tile([P, H, CW], f32r, tag="woc")
                        nc.sync.dma_start(
                            wot[:], wo[:, cs].rearrange("(o p) n -> p o n", p=P))
                        for m in range(DO):
                            ps = fps.tile([P, CW], f32, tag="f")
                            for hh in range(H):
                                nc.tensor.matmul(
                                    ps[:], oT[:, hh, m * P:(m + 1) * P],
                                    wot[:, hh, :], start=(hh == 0),
                                    stop=(hh == H - 1 and not with_bias))
                            if with_bias:
                                nc.tensor.matmul(
                                    ps[:], ones1[:], bo_t[:, cs],
                                    start=False, stop=True)
                            ot = fop.tile([P, CW], f32r, tag="fo")
                            nc.scalar.activation(ot[:], ps[:], COPY)
                            nc.gpsimd.dma_start(
                                out_d[m * P:(m + 1) * P, cs], ot[:])
    return _split_excess_waits(nc)


def kernel(queries, keys, values, attention_mask, Wq, bq, Wk, bk, Wv, bv,
           Wo, bo, m_k, m_v):
    from concourse.bass_utils import run_bass_kernel_spmd

    queries = np.asarray(queries, dtype=np.float32)
    keys = np.asarray(keys, dtype=np.float32)
    values = np.asarray(values, dtype=np.float32)
    mask = np.asarray(attention_mask)
    Wq, Wk, Wv, Wo = (np.asarray(w, dtype=np.float32) for w in (Wq, Wk, Wv, Wo))
    bq, bk, bv, bo = (np.asarray(b, dtype=np.float32) for b in (bq, bk, bv, bo))
    m_k = np.asarray(m_k, dtype=np.float32)
    m_v = np.asarray(m_v, dtype=np.float32)

    with_bias = any(np.any(b) for b in (bq, bk, bv, bo))
    key = ("prog", with_bias)
    if key not in _cached:
        _cached[key] = _build_program(with_bias)
    nc = _cached[key]

    mkT_all = np.ascontiguousarray(
        (math.sqrt(D_K) * m_k[0].T).reshape(H, D_K, M))
    mv_all = np.ascontiguousarray(math.sqrt(M) * m_v[0])
    notmask = np.logical_not(mask)

    in_maps = []
    for b in range(B):
        im = {
            "xqT": np.ascontiguousarray(queries[b].T),
            "xkT": np.ascontiguousarray(keys[b].T),
            "xvT": np.ascontiguousarray(values[b].T),
            "wq": Wq, "wk": Wk, "wv": Wv, "wo": Wo,
            "mkT": mkT_all, "mv": mv_all,
            "nmT": np.ascontiguousarray(
                notmask[b].transpose(0, 2, 1)).view(np.uint8),
        }
        if with_bias:
            im["bq"] = np.ascontiguousarray(bq.reshape(DO, P).T)
            im["bk"] = np.ascontiguousarray(bk.reshape(DO, P).T)
            im["bv"] = bv.reshape(1, H * D_V)
            im["bo"] = bo.reshape(1, D_MODEL)
        in_maps.append(im)

    res = run_bass_kernel_spmd(nc, in_maps, list(range(B)))

    out = np.empty((B, NQ, D_MODEL), dtype=np.float32)
    att = np.empty((B, H, NQ, NKM), dtype=np.float32)
    for b in range(B):
        r = res.results[b]
        out[b] = r["out"]
        np.divide(r["attT"].transpose(0, 2, 1), r["sums"][:, :, None],
                  out=att[b])
    return out, att.reshape(-1, NQ, NK)


# revision 24
# speedup vs baseline: 1.0311x; 1.0101x over previous
"""Augmented-memory scaled-dot-product attention on 8 Trainium2 NeuronCores.

Sharding: data-parallel over batch (B=8 -> 1 batch element per core), weights
replicated.  Inside each core everything is computed in a transposed layout:

  qT[hd, q]   = Wq^T @ x_q^T          (projections, contraction over d_model)
  kT[hd, k]   = Wk^T @ x_k^T  (+ memory slots appended as columns)
  v[k, hd]    = x_v^T as lhsT @ Wv    (natural layout, via DRAM, memory slots
                                       kept on-chip)
  sT[k, q]    = kT_h-slice^T @ qT_h   (scores, transposed: k on partitions)
  eT[k, q]    = exp(scale * sT) * notmask          (masked unnormalized)
  sums[q]     = ones^T @ eT           (softmax denominator via PE ones-matmul)
  oT[dv, q]   = v_h-slice^T @ eT, normalized by broadcast(1/sums)
  out[q, d]   = oT-slice^T @ Wo (+ bo)

The attention-probability output is returned transposed+unnormalized
(eT and sums); the host divides during the transpose it has to do anyway.
All matmul operands use float32r (full PE rate; same bits as float32).
"""

import math

import numpy as np

B, NQ, NK, D_MODEL, H, D_K, D_V, M = 8, 1024, 1024, 1024, 8, 128, 128, 16
NKM = NK + M          # 1040
P = 128
DO = D_MODEL // P     # 8 d-tiles
KO = NK // P          # 8 full k-tiles
CH = 2                # nq chunks of 512
CW = 512
SCALE = 1.0 / math.sqrt(D_K)

_cached = {}


def _split_excess_waits(nc):
    """This walrus build rejects >1 sync-wait per instruction. Move excess
    SyncWaits onto same-engine NOPs inserted directly before the owner."""
    import concourse.mybir as mybir

    for f in nc.m.functions:
        for bb in f.blocks:
            out, changed = [], False
            for inst in bb.instructions:
                si = inst.sync_info
                if si is not None and si.on_wait and len(si.on_wait) > 1:
                    waits = list(si.on_wait)
                    for w in waits[:-1]:
                        out.append(
                            mybir.InstNoOp(
                                name=nc.get_next_instruction_name(),
                                sync_info=mybir.SyncInfo(on_wait=[w], on_update=[]),
                                bass_nofuse=True,
                                engine=inst.engine,
                            )
                        )
                    si.on_wait = waits[-1:]
                    changed = True
                out.append(inst)
            if changed:
                bb.instructions = out
    return nc


def _build_program(with_bias):
    import concourse.bass as bass
    import concourse.mybir as mybir
    import concourse.tile as tile

    f32 = mybir.dt.float32
    f32r = mybir.dt.float32r
    u8 = mybir.dt.uint8
    EXP = mybir.ActivationFunctionType.Exp
    COPY = mybir.ActivationFunctionType.Copy
    MUL = mybir.AluOpType.mult

    nc = bass.Bass()
    dp = nc.declare_dram_parameter
    xqT = dp("xqT", [D_MODEL, NQ], f32r, isOutput=False)
    xkT = dp("xkT", [D_MODEL, NK], f32r, isOutput=False)
    xvT = dp("xvT", [D_MODEL, NK], f32r, isOutput=False)
    wq = dp("wq", [D_MODEL, H * D_K], f32r, isOutput=False)
    wk = dp("wk", [D_MODEL, H * D_K], f32r, isOutput=False)
    wv = dp("wv", [D_MODEL, H * D_V], f32r, isOutput=False)
    wo = dp("wo", [H * D_V, D_MODEL], f32r, isOutput=False)
    mkT = dp("mkT", [H, D_K, M], f32r, isOutput=False)   # sqrt(dk)*m_k, [h][dk][slot]
    mv = dp("mv", [M, H * D_V], f32r, isOutput=False)    # sqrt(M)*m_v
    nmT = dp("nmT", [H, NK, NQ], u8, isOutput=False)     # ~mask, [h][k][q]
    if with_bias:
        bqp = dp("bq", [P, DO], f32, isOutput=False)     # partition-major
        bkp = dp("bk", [P, DO], f32, isOutput=False)
        bvp = dp("bv", [1, H * D_V], f32r, isOutput=False)
        bop = dp("bo", [1, D_MODEL], f32r, isOutput=False)
    out_d = dp("out", [NQ, D_MODEL], f32r, isOutput=True)
    attT_d = dp("attT", [H, NKM, NQ], f32r, isOutput=True)
    sums_d = dp("sums", [H, NQ], f32r, isOutput=True)

    with tile.TileContext(nc) as tc:
        with tc.tile_pool(name="persist", bufs=1) as pers:
            qT = pers.tile([P, H, NQ], f32r, name="qT")
            kT = pers.tile([P, H, NKM], f32r, name="kT")
            vv = pers.tile([P, KO, H * D_V], f32r, name="vv")
            vmem = pers.tile([M, H * D_V], f32r, name="vmem")
            onesK = pers.tile([P, 1], f32r, name="onesK")
            onesM = pers.tile([M, 1], f32r, name="onesM")
            ones1 = pers.tile([1, P], f32r, name="ones1")
            ones_f = pers.tile([P, 1], f32, name="ones_f")
            nc.vector.memset(ones_f[:], 1.0)
            nc.vector.tensor_copy(onesK[:], ones_f[:])
            nc.vector.tensor_copy(onesM[:], ones_f[:M, :])
            one_row_f = pers.tile([1, P], f32, name="one_row_f")
            nc.vector.memset(one_row_f[:], 1.0)
            nc.vector.tensor_copy(ones1[:], one_row_f[:])
            if with_bias:
                bq_t = pers.tile([P, DO], f32, name="bq_t")
                bk_t = pers.tile([P, DO], f32, name="bk_t")
                bv_t = pers.tile([1, H * D_V], f32r, name="bv_t")
                bo_t = pers.tile([1, D_MODEL], f32r, name="bo_t")
                nc.sync.dma_start(bq_t[:], bqp[:])
                nc.sync.dma_start(bk_t[:], bkp[:])
                nc.sync.dma_start(bv_t[:], bvp[:])
                nc.sync.dma_start(bo_t[:], bop[:])

            # ------------- phase 1: projections (c-outer, x halves) -------------
            with tc.tile_pool(name="xpool", bufs=2) as xpool, \
                 tc.tile_pool(name="wpool", bufs=2) as wpool, \
                 tc.tile_pool(name="pjps", bufs=3, space="PSUM") as pjps:
                # for q/k the weight is stationary (full) and x streams in
                # column halves; for v the roles swap: xvT is stationary (full)
                # and Wv streams in hd-halves. Same indexing either way.
                for pi, (big, small) in enumerate(
                        [(wq, xqT), (wk, xkT), (xvT, wv)]):
                    wt = wpool.tile([P, DO, H * D_K], f32r, tag="w")
                    nc.sync.dma_start(wt[:], big.rearrange("(o p) n -> p o n", p=P))
                    for c in range(CH):
                        cs = slice(c * CW, (c + 1) * CW)
                        xt = xpool.tile([P, DO, CW], f32r, tag="x")
                        nc.sync.dma_start(
                            xt[:], small[:, cs].rearrange("(o p) n -> p o n", p=P))
                        for m in range(DO):
                            ps = pjps.tile([P, CW], f32, tag="pj")
                            for d in range(DO):
                                lhsT = wt[:, d, m * P:(m + 1) * P]
                                rhs = xt[:, d, :]
                                nc.tensor.matmul(
                                    ps[:], lhsT, rhs,
                                    start=(d == 0),
                                    stop=(d == DO - 1 and not (with_bias and pi == 2)))
                            if pi == 0:
                                bias = bq_t[:, m:m + 1] if with_bias else 0.0
                                nc.scalar.activation(
                                    qT[:, m, cs], ps[:], COPY, bias=bias)
                            elif pi == 1:
                                bias = bk_t[:, m:m + 1] if with_bias else 0.0
                                nc.scalar.activation(
                                    kT[:, m, cs], ps[:], COPY, bias=bias)
                            else:
                                if with_bias:
                                    nc.tensor.matmul(
                                        ps[:], ones1[:], bv_t[:, cs],
                                        start=False, stop=True)
                                nc.vector.tensor_copy(vv[:, m, cs], ps[:])
                # memory slots
                for h in range(H):
                    nc.sync.dma_start(kT[:, h, NK:NKM], mkT[h])
                nc.sync.dma_start(vmem[:], mv[:])

            # ---------------- phase 2: per-head attention ----------------
            with tc.tile_pool(name="otp", bufs=1) as otp:
                oT = otp.tile([P, H, NQ], f32r, name="oT")
                with tc.tile_pool(name="expp", bufs=11) as expp, \
                     tc.tile_pool(name="emp", bufs=2) as emp, \
                     tc.tile_pool(name="nmp", bufs=2) as nmp, \
                     tc.tile_pool(name="rcp", bufs=1) as rcp, \
                     tc.tile_pool(name="smp", bufs=2) as smp, \
                     tc.tile_pool(name="sps", bufs=3, space="PSUM") as sps, \
                     tc.tile_pool(name="bcps", bufs=1, space="PSUM") as bcps, \
                     tc.tile_pool(name="smps", bufs=2, space="PSUM") as smps, \
                     tc.tile_pool(name="avps", bufs=2, space="PSUM") as avps:
                    LAG = 3
                    NS = KO + 1          # 8 k-tiles + memory-slot tile
                    st = {}              # per-head pipeline state

                    def finish(hp):
                        s = st[hp]
                        for c in range(CH):
                            cs = slice(c * CW, (c + 1) * CW)
                            bps = bcps.tile([P, CW], f32, tag="bc")
                            nc.tensor.matmul(
                                bps[:], ones1[:], s["lnt"][:, cs],
                                start=True, stop=True)
                            rc0 = rcp.tile([P, CW], f32, tag="rc0")
                            nc.scalar.activation(rc0[:], bps[:], EXP, scale=-1.0)
                            # one Newton-Raphson step, sign-folded against the
                            # negated oT eviction: oT_final = oT_u / sums
                            bps2 = bcps.tile([P, CW], f32, tag="bc")
                            nc.tensor.matmul(
                                bps2[:], ones1[:], s["sums_sb"][:, cs],
                                start=True, stop=True)
                            u = rcp.tile([P, CW], f32, tag="u")
                            nc.vector.tensor_tensor(u[:], bps2[:], rc0[:], MUL)
                            nc.vector.scalar_tensor_tensor(
                                out=u[:], in0=u[:], scalar=2.0, in1=rc0[:],
                                op0=mybir.AluOpType.subtract, op1=MUL)
                            nc.vector.tensor_tensor(
                                oT[:, hp, cs], oT[:, hp, cs], u[:], MUL)

                    def emit_score(h, t):
                        s = st[h]
                        if t < KO:
                            k = t
                            nmt = nmp.tile([P, NQ], u8, tag="nm",
                                           name=f"nm_{h}_{k}")
                            nc.sync.dma_start(
                                nmt[:], nmT[h, k * P:(k + 1) * P, :])
                            et = expp.tile([P, NQ], f32r, tag="exp",
                                           name=f"exp_{h}_{k}")
                            for c in range(CH):
                                cs = slice(c * CW, (c + 1) * CW)
                                ps = sps.tile([P, CW], f32, tag="s",
                                              name=f"s_{h}_{k}_{c}")
                                nc.tensor.matmul(
                                    ps[:], kT[:, h, k * P:(k + 1) * P],
                                    qT[:, h, cs], start=True, stop=True)
                                nc.scalar.activation(
                                    et[:, cs], ps[:], EXP, scale=SCALE)
                            nc.vector.tensor_tensor(et[:], et[:], nmt[:], MUL)
                            nc.gpsimd.dma_start(
                                attT_d[h, k * P:(k + 1) * P, :], et[:])
                            s["tiles"].append(et)
                        else:
                            em = emp.tile([M, NQ], f32r, tag="em", name=f"em_{h}")
                            for c in range(CH):
                                cs = slice(c * CW, (c + 1) * CW)
                                psm = sps.tile([M, CW], f32, tag="s",
                                               name=f"sm_{h}_{c}")
                                nc.tensor.matmul(
                                    psm[:], kT[:, h, NK:NKM], qT[:, h, cs],
                                    start=True, stop=True)
                                nc.scalar.activation(
                                    em[:, cs], psm[:], EXP, scale=SCALE)
                            nc.gpsimd.dma_start(attT_d[h, NK:NKM, :], em[:])
                            s["tiles"].append(em)

                    def emit_sumav(h, c, t):
                        s = st[h]
                        cs = slice(c * CW, (c + 1) * CW)
                        if t == 0:
                            s["sum"][c] = smps.tile([1, CW], f32, tag="sum",
                                                    name=f"sum_{h}_{c}")
                            s["av"][c] = avps.tile([P, CW], f32, tag="av",
                                                   name=f"av_{h}_{c}")
                        lhs_s = onesK if t < KO else onesM
                        lhs_v = (vv[:, t, h * P:(h + 1) * P] if t < KO
                                 else vmem[:, h * P:(h + 1) * P])
                        nc.tensor.matmul(
                            s["sum"][c][:], lhs_s[:], s["tiles"][t][:, cs],
                            start=(t == 0), stop=(t == NS - 1))
                        nc.tensor.matmul(
                            s["av"][c][:], lhs_v, s["tiles"][t][:, cs],
                            start=(t == 0), stop=(t == NS - 1))
                        if t == NS - 1:
                            nc.vector.tensor_copy(
                                s["sums_sb"][:, cs], s["sum"][c][:])
                            nc.scalar.activation(
                                s["lnt"][:, cs], s["sums_sb"][:, cs],
                                mybir.ActivationFunctionType.Ln)
                            nc.scalar.activation(
                                oT[:, h, cs], s["av"][c][:], COPY, scale=-1.0)
                            if c == 1:
                                nc.gpsimd.dma_start(
                                    sums_d[h:h + 1, :], s["sums_sb"][:])

                    # two-deep pipeline: head h's scores interleave with h's c0
                    # sums/av AND head h-1's c1 sums/av, so ACT exp production
                    # always has PE consumption alongside
                    for h in range(H):
                        st[h] = {"tiles": [], "sum": [None, None],
                                 "av": [None, None],
                                 "sums_sb": smp.tile([1, NQ], f32r, tag="sumsb",
                                                     name=f"ssb_{h}"),
                                 "lnt": smp.tile([1, NQ], f32r, tag="lnt",
                                                 name=f"lnt_{h}")}
                        for t in range(NS):
                            emit_score(h, t)
                            if h > 0:
                                emit_sumav(h - 1, 1, t)
                            if t >= LAG:
                                emit_sumav(h, 0, t - LAG)
                        for t in range(NS - LAG, NS):
                            emit_sumav(h, 0, t)
                        if h > 0:
                            finish(h - 1)
                            del st[h - 1]
                    for t in range(NS):
                        emit_sumav(H - 1, 1, t)
                    finish(H - 1)

                # ------------ phase 3: output projection ------------
                with tc.tile_pool(name="wop", bufs=2) as wop, \
                     tc.tile_pool(name="fop", bufs=3) as fop, \
                     tc.tile_pool(name="fps", bufs=2, space="PSUM") as fps:
                    for c in range(CH):
                        cs = slice(c * CW, (c + 1) * CW)
                        wot = wop.tile([P, H, CW], f32r, tag="woc")
                        nc.sync.dma_start(
                            wot[:], wo[:, cs].rearrange("(o p) n -> p o n", p=P))
                        for m in range(DO):
                            ps = fps.tile([P, CW], f32, tag="f")
                            for hh in range(H):
                                nc.tensor.matmul(
                                    ps[:], oT[:, hh, m * P:(m + 1) * P],
                                    wot[:, hh, :], start=(hh == 0),
                                    stop=(hh == H - 1 and not with_bias))
                            if with_bias:
                                nc.tensor.matmul(
                                    ps[:], ones1[:], bo_t[:, cs],
                                    start=False, stop=True)
                            ot = fop.tile([P, CW], f32r, tag="fo")
                            nc.scalar.activation(ot[:], ps[:], COPY)
                            nc.gpsimd.dma_start(
                                out_d[m * P:(m + 1) * P, cs], ot[:])
    return _split_excess_waits(nc)


def kernel(queries, keys, values, attention_mask, Wq, bq, Wk, bk, Wv, bv,
           Wo, bo, m_k, m_v):
    from concourse.bass_utils import run_bass_kernel_spmd

    queries = np.asarray(queries, dtype=np.float32)
    keys = np.asarray(keys, dtype=np.float32)
    values = np.asarray(values, dtype=np.float32)
    mask = np.asarray(attention_mask)
    Wq, Wk, Wv, Wo = (np.asarray(w, dtype=np.float32) for w in (Wq, Wk, Wv, Wo))
    bq, bk, bv, bo = (np.asarray(b, dtype=np.float32) for b in (bq, bk, bv, bo))
    m_k = np.asarray(m_k, dtype=np.float32)
    m_v = np.asarray(m_v, dtype=np.float32)

    with_bias = any(np.any(b) for b in (bq, bk, bv, bo))
    key = ("prog", with_bias)
    if key not in _cached:
        _cached[key] = _build_program(with_bias)
    nc = _cached[key]

    mkT_all = np.ascontiguousarray(
        (math.sqrt(D_K) * m_k[0].T).reshape(H, D_K, M))
    mv_all = np.ascontiguousarray(math.sqrt(M) * m_v[0])
    notmask = np.logical_not(mask)

    in_maps = []
    for b in range(B):
        im = {
            "xqT": np.ascontiguousarray(queries[b].T),
            "xkT": np.ascontiguousarray(keys[b].T),
            "xvT": np.ascontiguousarray(values[b].T),
            "wq": Wq, "wk": Wk, "wv": Wv, "wo": Wo,
            "mkT": mkT_all, "mv": mv_all,
            "nmT": np.ascontiguousarray(
                notmask[b].transpose(0, 2, 1)).view(np.uint8),
        }
        if with_bias:
            im["bq"] = np.ascontiguousarray(bq.reshape(DO, P).T)
            im["bk"] = np.ascontiguousarray(bk.reshape(DO, P).T)
            im["bv"] = bv.reshape(1, H * D_V)
            im["bo"] = bo.reshape(1, D_MODEL)
        in_maps.append(im)

    res = run_bass_kernel_spmd(nc, in_maps, list(range(B)))

    out = np.empty((B, NQ, D_MODEL), dtype=np.float32)
    att = np.empty((B, H, NQ, NKM), dtype=np.float32)
    for b in range(B):
        r = res.results[b]
        out[b] = r["out"]
        np.divide(r["attT"].transpose(0, 2, 1), r["sums"][:, :, None],
                  out=att[b])
    return out, att.reshape(-1, NQ, NK)


# revision 25
# speedup vs baseline: 1.0824x; 1.0497x over previous
"""Augmented-memory scaled-dot-product attention on 8 Trainium2 NeuronCores.

Sharding: data-parallel over batch (B=8 -> 1 batch element per core), weights
replicated.  Inside each core everything is computed in a transposed layout:

  qT[hd, q]   = Wq^T @ x_q^T          (projections, contraction over d_model)
  kT[hd, k]   = Wk^T @ x_k^T  (+ memory slots appended as columns)
  v[k, hd]    = x_v^T as lhsT @ Wv    (natural layout, via DRAM, memory slots
                                       kept on-chip)
  sT[k, q]    = kT_h-slice^T @ qT_h   (scores, transposed: k on partitions)
  eT[k, q]    = exp(scale * sT) * notmask          (masked unnormalized)
  sums[q]     = ones^T @ eT           (softmax denominator via PE ones-matmul)
  oT[dv, q]   = v_h-slice^T @ eT, normalized by broadcast(1/sums)
  out[q, d]   = oT-slice^T @ Wo (+ bo)

The attention-probability output is returned transposed+unnormalized
(eT and sums); the host divides during the transpose it has to do anyway.
All matmul operands use float32r (full PE rate; same bits as float32).
"""

import math

import numpy as np

B, NQ, NK, D_MODEL, H, D_K, D_V, M = 8, 1024, 1024, 1024, 8, 128, 128, 16
NKM = NK + M          # 1040
P = 128
DO = D_MODEL // P     # 8 d-tiles
KO = NK // P          # 8 full k-tiles
CH = 2                # nq chunks of 512
CW = 512
SCALE = 1.0 / math.sqrt(D_K)

_cached = {}


def _split_excess_waits(nc):
    """This walrus build rejects >1 sync-wait per instruction. Move excess
    SyncWaits onto same-engine NOPs inserted directly before the owner."""
    import concourse.mybir as mybir

    for f in nc.m.functions:
        for bb in f.blocks:
            out, changed = [], False
            for inst in bb.instructions:
                si = inst.sync_info
                if si is not None and si.on_wait and len(si.on_wait) > 1:
                    waits = list(si.on_wait)
                    for w in waits[:-1]:
                        out.append(
                            mybir.InstNoOp(
                                name=nc.get_next_instruction_name(),
                                sync_info=mybir.SyncInfo(on_wait=[w], on_update=[]),
                                bass_nofuse=True,
                                engine=inst.engine,
                            )
                        )
                    si.on_wait = waits[-1:]
                    changed = True
                out.append(inst)
            if changed:
                bb.instructions = out
    return nc


def _build_program(with_bias):
    import concourse.bass as bass
    import concourse.mybir as mybir
    import concourse.tile as tile

    f32 = mybir.dt.float32
    f32r = mybir.dt.float32r
    u8 = mybir.dt.uint8
    EXP = mybir.ActivationFunctionType.Exp
    COPY = mybir.ActivationFunctionType.Copy
    MUL = mybir.AluOpType.mult

    nc = bass.Bass()
    dp = nc.declare_dram_parameter
    xqT = dp("xqT", [D_MODEL, NQ], f32r, isOutput=False)
    xkT = dp("xkT", [D_MODEL, NK], f32r, isOutput=False)
    xvT = dp("xvT", [D_MODEL, NK], f32r, isOutput=False)
    wq = dp("wq", [D_MODEL, H * D_K], f32r, isOutput=False)
    wk = dp("wk", [D_MODEL, H * D_K], f32r, isOutput=False)
    wv = dp("wv", [D_MODEL, H * D_V], f32r, isOutput=False)
    wo = dp("wo", [H * D_V, D_MODEL], f32r, isOutput=False)
    mkT = dp("mkT", [H, D_K, M], f32r, isOutput=False)   # sqrt(dk)*m_k, [h][dk][slot]
    mv = dp("mv", [M, H * D_V], f32r, isOutput=False)    # sqrt(M)*m_v
    nmT = dp("nmT", [H, NK, NQ], u8, isOutput=False)     # ~mask, [h][k][q]
    if with_bias:
        bqp = dp("bq", [P, DO], f32, isOutput=False)     # partition-major
        bkp = dp("bk", [P, DO], f32, isOutput=False)
        bvp = dp("bv", [1, H * D_V], f32r, isOutput=False)
        bop = dp("bo", [1, D_MODEL], f32r, isOutput=False)
    out_d = dp("out", [NQ, D_MODEL], f32r, isOutput=True)
    attT_d = dp("attT", [H, NKM, NQ], f32r, isOutput=True)
    sums_d = dp("sums", [H, NQ], f32r, isOutput=True)

    with tile.TileContext(nc) as tc:
        with tc.tile_pool(name="persist", bufs=1) as pers:
            qT = pers.tile([P, H, NQ], f32r, name="qT")
            kT = pers.tile([P, H, NKM], f32r, name="kT")
            vv = pers.tile([P, KO, H * D_V], f32r, name="vv")
            vmem = pers.tile([M, H * D_V], f32r, name="vmem")
            onesK = pers.tile([P, 1], f32r, name="onesK")
            onesM = pers.tile([M, 1], f32r, name="onesM")
            ones1 = pers.tile([1, P], f32r, name="ones1")
            ones_f = pers.tile([P, 1], f32, name="ones_f")
            nc.vector.memset(ones_f[:], 1.0)
            nc.vector.tensor_copy(onesK[:], ones_f[:])
            nc.vector.tensor_copy(onesM[:], ones_f[:M, :])
            one_row_f = pers.tile([1, P], f32, name="one_row_f")
            nc.vector.memset(one_row_f[:], 1.0)
            nc.vector.tensor_copy(ones1[:], one_row_f[:])
            if with_bias:
                bq_t = pers.tile([P, DO], f32, name="bq_t")
                bk_t = pers.tile([P, DO], f32, name="bk_t")
                bv_t = pers.tile([1, H * D_V], f32r, name="bv_t")
                bo_t = pers.tile([1, D_MODEL], f32r, name="bo_t")
                nc.sync.dma_start(bq_t[:], bqp[:])
                nc.sync.dma_start(bk_t[:], bkp[:])
                nc.sync.dma_start(bv_t[:], bvp[:])
                nc.sync.dma_start(bo_t[:], bop[:])

            # ------------- phase 1: projections (c-outer, x halves) -------------
            with tc.tile_pool(name="xpool", bufs=2) as xpool, \
                 tc.tile_pool(name="wpool", bufs=2) as wpool, \
                 tc.tile_pool(name="pjps", bufs=3, space="PSUM") as pjps:
                # for q/k the weight is stationary (full) and x streams in
                # column halves; for v the roles swap: xvT is stationary (full)
                # and Wv streams in hd-halves. Same indexing either way.
                for pi, (big, small) in enumerate(
                        [(wq, xqT), (wk, xkT), (xvT, wv)]):
                    wt = wpool.tile([P, DO, H * D_K], f32r, tag="w")
                    nc.sync.dma_start(wt[:], big.rearrange("(o p) n -> p o n", p=P))
                    for c in range(CH):
                        cs = slice(c * CW, (c + 1) * CW)
                        xt = xpool.tile([P, DO, CW], f32r, tag="x")
                        nc.sync.dma_start(
                            xt[:], small[:, cs].rearrange("(o p) n -> p o n", p=P))
                        for m in range(DO):
                            ps = pjps.tile([P, CW], f32, tag="pj")
                            for d in range(DO):
                                lhsT = wt[:, d, m * P:(m + 1) * P]
                                rhs = xt[:, d, :]
                                nc.tensor.matmul(
                                    ps[:], lhsT, rhs,
                                    start=(d == 0),
                                    stop=(d == DO - 1 and not (with_bias and pi == 2)))
                            if pi == 0:
                                bias = bq_t[:, m:m + 1] if with_bias else 0.0
                                nc.scalar.activation(
                                    qT[:, m, cs], ps[:], COPY, bias=bias)
                            elif pi == 1:
                                bias = bk_t[:, m:m + 1] if with_bias else 0.0
                                nc.scalar.activation(
                                    kT[:, m, cs], ps[:], COPY, bias=bias)
                            else:
                                if with_bias:
                                    nc.tensor.matmul(
                                        ps[:], ones1[:], bv_t[:, cs],
                                        start=False, stop=True)
                                nc.vector.tensor_copy(vv[:, m, cs], ps[:])
                # memory slots
                for h in range(H):
                    nc.sync.dma_start(kT[:, h, NK:NKM], mkT[h])
                nc.sync.dma_start(vmem[:], mv[:])

            # ---------------- phase 2: per-head attention ----------------
            with tc.tile_pool(name="otp", bufs=1) as otp:
                oT = otp.tile([P, H, NQ], f32r, name="oT")
                with tc.tile_pool(name="expp", bufs=11) as expp, \
                     tc.tile_pool(name="emp", bufs=2) as emp, \
                     tc.tile_pool(name="nmp", bufs=2) as nmp, \
                     tc.tile_pool(name="rcp", bufs=1) as rcp, \
                     tc.tile_pool(name="smp", bufs=2) as smp, \
                     tc.tile_pool(name="sps", bufs=3, space="PSUM") as sps, \
                     tc.tile_pool(name="bcps", bufs=1, space="PSUM") as bcps, \
                     tc.tile_pool(name="smps", bufs=2, space="PSUM") as smps, \
                     tc.tile_pool(name="avps", bufs=2, space="PSUM") as avps:
                    LAG = 4
                    NS = KO + 1          # 8 k-tiles + memory-slot tile
                    st = {}              # per-head pipeline state

                    def finish(hp):
                        s = st[hp]
                        for c in range(CH):
                            cs = slice(c * CW, (c + 1) * CW)
                            bps = bcps.tile([P, CW], f32, tag="bc")
                            nc.tensor.matmul(
                                bps[:], ones1[:], s["lnt"][:, cs],
                                start=True, stop=True)
                            rc0 = rcp.tile([P, CW], f32, tag="rc0")
                            nc.scalar.activation(rc0[:], bps[:], EXP, scale=-1.0)
                            # one Newton-Raphson step, sign-folded against the
                            # negated oT eviction: oT_final = oT_u / sums
                            bps2 = bcps.tile([P, CW], f32, tag="bc")
                            nc.tensor.matmul(
                                bps2[:], ones1[:], s["sums_sb"][:, cs],
                                start=True, stop=True)
                            u = rcp.tile([P, CW], f32, tag="u")
                            nc.vector.tensor_tensor(u[:], bps2[:], rc0[:], MUL)
                            nc.vector.scalar_tensor_tensor(
                                out=u[:], in0=u[:], scalar=2.0, in1=rc0[:],
                                op0=mybir.AluOpType.subtract, op1=MUL)
                            nc.vector.tensor_tensor(
                                oT[:, hp, cs], oT[:, hp, cs], u[:], MUL)

                    def emit_score(h, t):
                        s = st[h]
                        if t < KO:
                            k = t
                            nmt = nmp.tile([P, NQ], u8, tag="nm",
                                           name=f"nm_{h}_{k}")
                            nc.sync.dma_start(
                                nmt[:], nmT[h, k * P:(k + 1) * P, :])
                            et = expp.tile([P, NQ], f32r, tag="exp",
                                           name=f"exp_{h}_{k}")
                            for c in range(CH):
                                cs = slice(c * CW, (c + 1) * CW)
                                ps = sps.tile([P, CW], f32, tag="s",
                                              name=f"s_{h}_{k}_{c}")
                                nc.tensor.matmul(
                                    ps[:], kT[:, h, k * P:(k + 1) * P],
                                    qT[:, h, cs], start=True, stop=True)
                                nc.scalar.activation(
                                    et[:, cs], ps[:], EXP, scale=SCALE)
                            nc.vector.tensor_tensor(et[:], et[:], nmt[:], MUL)
                            nc.gpsimd.dma_start(
                                attT_d[h, k * P:(k + 1) * P, :], et[:])
                            s["tiles"].append(et)
                        else:
                            em = emp.tile([M, NQ], f32r, tag="em", name=f"em_{h}")
                            for c in range(CH):
                                cs = slice(c * CW, (c + 1) * CW)
                                psm = sps.tile([M, CW], f32, tag="s",
                                               name=f"sm_{h}_{c}")
                                nc.tensor.matmul(
                                    psm[:], kT[:, h, NK:NKM], qT[:, h, cs],
                                    start=True, stop=True)
                                nc.scalar.activation(
                                    em[:, cs], psm[:], EXP, scale=SCALE)
                            nc.gpsimd.dma_start(attT_d[h, NK:NKM, :], em[:])
                            s["tiles"].append(em)

                    def emit_sumav(h, c, t):
                        s = st[h]
                        cs = slice(c * CW, (c + 1) * CW)
                        if t == 0:
                            s["sum"][c] = smps.tile([1, CW], f32, tag="sum",
                                                    name=f"sum_{h}_{c}")
                            s["av"][c] = avps.tile([P, CW], f32, tag="av",
                                                   name=f"av_{h}_{c}")
                        lhs_s = onesK if t < KO else onesM
                        lhs_v = (vv[:, t, h * P:(h + 1) * P] if t < KO
                                 else vmem[:, h * P:(h + 1) * P])
                        nc.tensor.matmul(
                            s["sum"][c][:], lhs_s[:], s["tiles"][t][:, cs],
                            start=(t == 0), stop=(t == NS - 1))
                        nc.tensor.matmul(
                            s["av"][c][:], lhs_v, s["tiles"][t][:, cs],
                            start=(t == 0), stop=(t == NS - 1))
                        if t == NS - 1:
                            nc.vector.tensor_copy(
                                s["sums_sb"][:, cs], s["sum"][c][:])
                            nc.scalar.activation(
                                s["lnt"][:, cs], s["sums_sb"][:, cs],
                                mybir.ActivationFunctionType.Ln)
                            nc.scalar.activation(
                                oT[:, h, cs], s["av"][c][:], COPY, scale=-1.0)
                            if c == 1:
                                nc.gpsimd.dma_start(
                                    sums_d[h:h + 1, :], s["sums_sb"][:])

                    # two-deep pipeline: head h's scores interleave with h's c0
                    # sums/av AND head h-1's c1 sums/av, so ACT exp production
                    # always has PE consumption alongside
                    for h in range(H):
                        st[h] = {"tiles": [], "sum": [None, None],
                                 "av": [None, None],
                                 "sums_sb": smp.tile([1, NQ], f32r, tag="sumsb",
                                                     name=f"ssb_{h}"),
                                 "lnt": smp.tile([1, NQ], f32r, tag="lnt",
                                                 name=f"lnt_{h}")}
                        for t in range(NS):
                            emit_score(h, t)
                            if h > 0:
                                emit_sumav(h - 1, 1, t)
                            if t >= LAG:
                                emit_sumav(h, 0, t - LAG)
                        for t in range(NS - LAG, NS):
                            emit_sumav(h, 0, t)
                        if h > 0:
                            finish(h - 1)
                            del st[h - 1]
                    for t in range(NS):
                        emit_sumav(H - 1, 1, t)
                    finish(H - 1)

                # ------------ phase 3: output projection ------------
                with tc.tile_pool(name="wop", bufs=2) as wop, \
                     tc.tile_pool(name="fop", bufs=3) as fop, \
                     tc.tile_pool(name="fps", bufs=2, space="PSUM") as fps:
                    for c in range(CH):
                        cs = slice(c * CW, (c + 1) * CW)
                        wot = wop.tile([P, H, CW], f32r, tag="woc")
                        nc.sync.dma_start(
                            wot[:], wo[:, cs].rearrange("(o p) n -> p o n", p=P))
                        for m in range(DO):
                            ps = fps.tile([P, CW], f32, tag="f")
                            for hh in range(H):
                                nc.tensor.matmul(
                                    ps[:], oT[:, hh, m * P:(m + 1) * P],
                                    wot[:, hh, :], start=(hh == 0),
                                    stop=(hh == H - 1 and not with_bias))
                            if with_bias:
                                nc.tensor.matmul(
                                    ps[:], ones1[:], bo_t[:, cs],
                                    start=False, stop=True)
                            ot = fop.tile([P, CW], f32r, tag="fo")
                            nc.scalar.activation(ot[:], ps[:], COPY)
                            nc.gpsimd.dma_start(
                                out_d[m * P:(m + 1) * P, cs], ot[:])
    return _split_excess_waits(nc)


def kernel(queries, keys, values, attention_mask, Wq, bq, Wk, bk, Wv, bv,
           Wo, bo, m_k, m_v):
    from concourse.bass_utils import run_bass_kernel_spmd

    queries = np.asarray(queries, dtype=np.float32)
    keys = np.asarray(keys, dtype=np.float32)
    values = np.asarray(values, dtype=np.float32)
    mask = np.asarray(attention_mask)
    Wq, Wk, Wv, Wo = (np.asarray(w, dtype=np.float32) for w in (Wq, Wk, Wv, Wo))
    bq, bk, bv, bo = (np.asarray(b, dtype=np.float32) for b in (bq, bk, bv, bo))
    m_k = np.asarray(m_k, dtype=np.float32)
    m_v = np.asarray(m_v, dtype=np.float32)

    with_bias = any(np.any(b) for b in (bq, bk, bv, bo))
    key = ("prog", with_bias)
    if key not in _cached:
        _cached[key] = _build_program(with_bias)
    nc = _cached[key]

    mkT_all = np.ascontiguousarray(
        (math.sqrt(D_K) * m_k[0].T).reshape(H, D_K, M))
    mv_all = np.ascontiguousarray(math.sqrt(M) * m_v[0])
    notmask = np.logical_not(mask)

    in_maps = []
    for b in range(B):
        im = {
            "xqT": np.ascontiguousarray(queries[b].T),
            "xkT": np.ascontiguousarray(keys[b].T),
            "xvT": np.ascontiguousarray(values[b].T),
            "wq": Wq, "wk": Wk, "wv": Wv, "wo": Wo,
            "mkT": mkT_all, "mv": mv_all,
            "nmT": np.ascontiguousarray(
                notmask[b].transpose(0, 2, 1)).view(np.uint8),
        }
        if with_bias:
            im["bq"] = np.ascontiguousarray(bq.reshape(DO, P).T)
            im["bk"] = np.ascontiguousarray(bk.reshape(DO, P).T)
            im["bv"] = bv.reshape(1, H * D_V)
            im["bo"] = bo.reshape(1, D_MODEL)
        in_maps.append(im)

    res = run_bass_kernel_spmd(nc, in_maps, list(range(B)))

    out = np.empty((B, NQ, D_MODEL), dtype=np.float32)
    att = np.empty((B, H, NQ, NKM), dtype=np.float32)
    for b in range(B):
        r = res.results[b]
        out[b] = r["out"]
        np.divide(r["attT"].transpose(0, 2, 1), r["sums"][:, :, None],
                  out=att[b])
    return out, att.reshape(-1, NQ, NK)
